# revision 8
# baseline (speedup 1.0000x reference)
"""Trainium2 Bass kernel for nn_C4ByteNibbleVM (v3d: PE extraction + ACT gen).

Inputs are uploaded transposed + fp8 (one-hot along rows): aT[c_row, word]
with c_row = byte*256 + c.  The TensorEngine extracts nibble indices:
for each (byte, half) plane the data slice [128 c-rows, 128 words] is the
STATIONARY operand and a tiny iota [128, 2] (nib_lo(c), nib_hi(c)) is the
MOVING operand; psum[word, 2] accumulates the two halves -> exact
(lo_nib, hi_nib) per word-byte.  DVE does ripple-carry add + xor on byte
indices.  One-hot generation splits across engines: DVE is_equal (bf16,
4x mode, cast-stored to fp8) for half the word-chunks; for the other half
DVE computes d = iota - x per segment and ACT runs ONE whole-region
Square plus ONE Relu(1-sq) producing fp8 directly (batching the ACT work
into two big ops -- per-segment ACT ops are ~500 ns each and would
dominate).  Per core HBM traffic: 8+8 MB fp8 loads + 8 MB fp8 store
~= the per-core HBM roofline at ~358 GB/s.
"""

import numpy as np
import ml_dtypes

import concourse.bacc as bacc
import concourse.mybir as mybir
from concourse.tile import TileContext
from concourse import bass_utils

B = 65536
NCORES = 8
BLOC = B // NCORES          # words per core
W = 8                       # 128-word chunks per iteration (1024 words)
ROWS_PER_ITER = 128 * W
NITER = BLOC // (128 * W)

F32 = mybir.dt.float32
BF16 = mybir.dt.bfloat16
FP8 = mybir.dt.float8e4
I32 = mybir.dt.int32
AX = mybir.AxisListType
OP = mybir.AluOpType


def build_kernel(n_words=BLOC, w=W, reps=1, act_chunks=4):
    rows_per_iter = 128 * w
    n_iter = n_words // rows_per_iter
    fd = 1024 * w  # one-hot free dim of one iteration (words*4*256 bytes)
    nseg = 4 * w   # (wchunk, byte) segments per iteration
    dve_w = w - act_chunks          # wchunks generated on DVE (bf16)
    ACT = mybir.ActivationFunctionType

    nc = bacc.Bacc("TRN2", target_bir_lowering=False, debug=False)
    # transposed one-hot inputs: row = byte*256 + (128*h + r), col = word
    a_d = nc.dram_tensor("a", [1024, n_words], FP8, kind="ExternalInput")
    b_d = nc.dram_tensor("b", [1024, n_words], FP8, kind="ExternalInput")
    # moving iota: [128, (h, 2)] cols (lo, hi) per half
    iotam_d = nc.dram_tensor("iotam", [128, 4], FP8, kind="ExternalInput")
    iota_d = nc.dram_tensor("iota", [128, 256], BF16, kind="ExternalInput")
    y_d = nc.dram_tensor("y", [n_words, 1024], FP8, kind="ExternalOutput")

    # input views: [plane(byte,h), 128 c-rows, word]
    a_v = a_d[:].rearrange("(pl r) w -> pl r w", r=128)
    b_v = b_d[:].rearrange("(pl r) w -> pl r w", r=128)
    y_v = y_d[:].rearrange("(t s p) c -> t p s c", s=w, p=128)

    with TileContext(nc) as tc:
        with (
            tc.tile_pool(name="cst", bufs=1) as cst,
            tc.tile_pool(name="ld", bufs=2) as ld,
            tc.tile_pool(name="ps", bufs=2, space="PSUM") as psp,
            tc.tile_pool(name="idx", bufs=2) as idxp,
            tc.tile_pool(name="sm", bufs=2) as sm,
            tc.tile_pool(name="out", bufs=2) as outp,
        ):
            iotam_sb = cst.tile([128, 4], FP8)
            nc.gpsimd.dma_start(iotam_sb[:], iotam_d[:])
            iota_sb = cst.tile([128, 256], BF16)
            nc.gpsimd.dma_start(iota_sb[:], iota_d[:])

            for t in [t for _ in range(reps) for t in range(n_iter)]:
                a_t = ld.tile([128, 8, rows_per_iter // 128 * 128], FP8, tag="a")
                nc.sync.dma_start(
                    a_t[:], a_v[:, :, t * rows_per_iter : (t + 1) * rows_per_iter]
                    .rearrange("pl r w -> r pl w")
                )
                b_t = ld.tile([128, 8, rows_per_iter], FP8, tag="b")
                nc.sync.dma_start(
                    b_t[:], b_v[:, :, t * rows_per_iter : (t + 1) * rows_per_iter]
                    .rearrange("pl r w -> r pl w")
                )

                # psum: [word_p, wchunk, tensor, byte, nib]
                ps = psp.tile([128, w, 2, 4, 2], F32, tag="ps")
                for k in range(w):
                    for ti, src in enumerate((a_t, b_t)):
                        for byte in range(4):
                            for h in range(2):
                                nc.tensor.matmul(
                                    ps[:, k, ti, byte, :],
                                    src[:, byte * 2 + h, k * 128 : (k + 1) * 128],
                                    iotam_sb[:, 2 * h : 2 * h + 2],
                                    start=(h == 0),
                                    stop=(h == 1),
                                )

                # evacuate psum once, then byte index = lo + 16*hi per tensor
                nib = idxp.tile([128, w, 2, 4, 2], F32, tag="nib")
                nc.vector.tensor_copy(nib[:], ps[:])
                idxa = idxp.tile([128, nseg], F32, tag="ia")
                nc.vector.scalar_tensor_tensor(
                    idxa[:].rearrange("p (k i) -> p k i", i=4),
                    nib[:, :, 0, :, 1], 16.0, nib[:, :, 0, :, 0],
                    OP.mult, OP.add,
                )
                idxb = idxp.tile([128, nseg], F32, tag="ib")
                nc.vector.scalar_tensor_tensor(
                    idxb[:].rearrange("p (k i) -> p k i", i=4),
                    nib[:, :, 1, :, 1], 16.0, nib[:, :, 1, :, 0],
                    OP.mult, OP.add,
                )

                # ripple-carry add over byte positions i=0..3 (i inner in col)
                def bslice(ap, i):
                    return ap.rearrange("p (s i) -> p i s", i=4)[:, i : i + 1, :]

                csum = idxp.tile([128, nseg], F32, tag="cs")
                carry = None
                for i in range(4):
                    t0 = sm.tile([128, w], F32, tag=f"t0{i}")
                    nc.vector.tensor_tensor(
                        t0[:].rearrange("p (i s) -> p i s", i=1),
                        bslice(idxa[:], i),
                        bslice(idxb[:], i),
                        OP.add,
                    )
                    if carry is not None:
                        nc.vector.tensor_tensor(t0[:], t0[:], carry[:], OP.add)
                    cnew = sm.tile([128, w], F32, tag=f"c{i}")
                    nc.vector.tensor_scalar(cnew[:], t0[:], 256.0, None, OP.is_ge)
                    nc.vector.scalar_tensor_tensor(
                        bslice(csum[:], i),
                        cnew[:].rearrange("p (i s) -> p i s", i=1),
                        -256.0,
                        t0[:].rearrange("p (i s) -> p i s", i=1),
                        OP.mult,
                        OP.add,
                    )
                    carry = cnew

                # xor with operand a (int32), back to f32 for compares
                s_i = sm.tile([128, nseg], I32, tag="si")
                nc.vector.tensor_copy(s_i[:], csum[:])
                a_i = sm.tile([128, nseg], I32, tag="ai")
                nc.vector.tensor_copy(a_i[:], idxa[:])
                x_i = sm.tile([128, nseg], I32, tag="xi")
                nc.vector.tensor_tensor(x_i[:], s_i[:], a_i[:], OP.bitwise_xor)
                x_f = sm.tile([128, nseg], F32, tag="xf")
                nc.vector.tensor_copy(x_f[:], x_i[:])

                # DVE: direct is_equal (bf16) for wchunks [0, dve_w).
                out_t = outp.tile([128, dve_w * 1024], BF16, tag="o")
                for j in range(4 * dve_w):
                    nc.vector.tensor_scalar(
                        out_t[:, j * 256 : (j + 1) * 256],
                        iota_sb[:, 0:256],
                        x_f[:, j : j + 1],
                        None,
                        OP.is_equal,
                    )
                nc.gpsimd.dma_start(
                    y_v[t][:, 0:dve_w],
                    out_t[:].rearrange("p (s c) -> p s c", c=1024),
                )
                # ACT wchunks: DVE prediff d = iota - x per seg, then one
                # whole-region Square and one Relu(1-sq) -> fp8 on ACT.
                if act_chunks:
                    d_t = outp.tile([128, act_chunks * 1024], BF16, tag="d")
                    for j in range(4 * dve_w, nseg):
                        jj = j - 4 * dve_w
                        nc.vector.tensor_scalar(
                            d_t[:, jj * 256 : (jj + 1) * 256],
                            iota_sb[:, 0:256],
                            x_f[:, j : j + 1],
                            None,
                            OP.subtract,
                        )
                    sq_t = outp.tile([128, act_chunks * 1024], BF16, tag="sq")
                    nc.scalar.activation(sq_t[:], d_t[:], ACT.Square)
                    oa_t = outp.tile([128, act_chunks * 1024], FP8, tag="oa")
                    nc.scalar.activation(
                        oa_t[:], sq_t[:], ACT.Relu, bias=1.0, scale=-1.0
                    )
                    nc.sync.dma_start(
                        y_v[t][:, dve_w:w],
                        oa_t[:].rearrange("p (s c) -> p s c", c=1024),
                    )

    nc.compile()
    return nc


_CACHED = {}


def _get_kernel(n_words=BLOC, w=W):
    key = (n_words, w)
    if key not in _CACHED:
        _CACHED[key] = build_kernel(n_words, w)
    return _CACHED[key]


def _iotam_tile():
    # cols [lo(h0), hi(h0), lo(h1), hi(h1)] for c = 128*h + r
    r = np.arange(128, dtype=np.float32)
    c0 = r
    c1 = 128 + r
    m = np.stack(
        [c0 % 16, c0 // 16, c1 % 16, c1 // 16], axis=1
    ).astype(np.float32)
    return m.astype(ml_dtypes.float8_e4m3)


def _iota256_tile():
    row = np.arange(256, dtype=np.float32)
    return np.broadcast_to(row, (128, 256)).astype(ml_dtypes.bfloat16)


def make_in_maps(a, b, w=W):
    """a, b: [B, 1024] float arrays -> per-core input dicts (transposed fp8)."""
    iotam = _iotam_tile()
    iota = _iota256_tile()
    maps = []
    for c in range(NCORES):
        asl = np.asarray(a[c * BLOC : (c + 1) * BLOC]).astype(ml_dtypes.float8_e4m3)
        bsl = np.asarray(b[c * BLOC : (c + 1) * BLOC]).astype(ml_dtypes.float8_e4m3)
        maps.append(
            {
                "a": np.ascontiguousarray(asl.T),
                "b": np.ascontiguousarray(bsl.T),
                "iotam": iotam,
                "iota": iota,
            }
        )
    return maps


def kernel(**inputs):
    a = np.asarray(inputs["a_bytes"], dtype=np.float32).reshape(B, 1024)
    b = np.asarray(inputs["b_bytes"], dtype=np.float32).reshape(B, 1024)
    nc = _get_kernel()
    in_maps = make_in_maps(a, b)
    res = bass_utils.run_bass_kernel_spmd(nc, in_maps, core_ids=list(range(NCORES)))
    out = np.concatenate(
        [res.results[c]["y"].astype(np.float32) for c in range(NCORES)], axis=0
    )
    return out.reshape(B, 4, 256)


# revision 10
# speedup vs baseline: 1.2792x; 1.2792x over previous
"""Trainium2 Bass kernel for nn_C4ByteNibbleVM (v3d: PE extraction + ACT gen).

Inputs are uploaded transposed + fp8 (one-hot along rows): aT[c_row, word]
with c_row = byte*256 + c.  The TensorEngine extracts nibble indices:
for each (byte, half) plane the data slice [128 c-rows, 128 words] is the
STATIONARY operand and a tiny iota [128, 2] (nib_lo(c), nib_hi(c)) is the
MOVING operand; psum[word, 2] accumulates the two halves -> exact
(lo_nib, hi_nib) per word-byte.  DVE does ripple-carry add + xor on byte
indices.  One-hot generation splits across engines: DVE is_equal (bf16,
4x mode, cast-stored to fp8) for half the word-chunks; for the other half
DVE computes d = iota - x per segment and ACT runs ONE whole-region
Square plus ONE Relu(1-sq) producing fp8 directly (batching the ACT work
into two big ops -- per-segment ACT ops are ~500 ns each and would
dominate).  Per core HBM traffic: 8+8 MB fp8 loads + 8 MB fp8 store
~= the per-core HBM roofline at ~358 GB/s.
"""

import numpy as np
import ml_dtypes

import concourse.bacc as bacc
import concourse.mybir as mybir
from concourse.tile import TileContext
from concourse import bass_utils

B = 65536
NCORES = 8
BLOC = B // NCORES          # words per core
W = 16                      # 128-word chunks per iteration (2048 words)
ROWS_PER_ITER = 128 * W
NITER = BLOC // (128 * W)

F32 = mybir.dt.float32
BF16 = mybir.dt.bfloat16
FP8 = mybir.dt.float8e4
I32 = mybir.dt.int32
AX = mybir.AxisListType
OP = mybir.AluOpType


def build_kernel(n_words=BLOC, w=W, reps=1, act_chunks=None):
    rows_per_iter = 128 * w
    n_iter = n_words // rows_per_iter
    fd = 1024 * w  # one-hot free dim of one iteration (words*4*256 bytes)
    nseg = 4 * w   # (wchunk, byte) segments per iteration
    if act_chunks is None:
        act_chunks = w // 2         # wchunks generated via ACT (fp8)
    dve_w = w - act_chunks          # wchunks generated on DVE (bf16)
    ACT = mybir.ActivationFunctionType

    nc = bacc.Bacc("TRN2", target_bir_lowering=False, debug=False)
    # transposed one-hot inputs: row = byte*256 + (128*h + r), col = word
    a_d = nc.dram_tensor("a", [1024, n_words], FP8, kind="ExternalInput")
    b_d = nc.dram_tensor("b", [1024, n_words], FP8, kind="ExternalInput")
    # moving iota: [128, (h, 2)] cols (lo, hi) per half
    iotam_d = nc.dram_tensor("iotam", [128, 4], FP8, kind="ExternalInput")
    iota_d = nc.dram_tensor("iota", [128, 256], BF16, kind="ExternalInput")
    y_d = nc.dram_tensor("y", [n_words, 1024], FP8, kind="ExternalOutput")

    # input views: [plane(byte,h), 128 c-rows, word]
    a_v = a_d[:].rearrange("(pl r) w -> pl r w", r=128)
    b_v = b_d[:].rearrange("(pl r) w -> pl r w", r=128)
    y_v = y_d[:].rearrange("(t s p) c -> t p s c", s=w, p=128)

    with TileContext(nc) as tc:
        with (
            tc.tile_pool(name="cst", bufs=1) as cst,
            tc.tile_pool(name="ld", bufs=2) as ld,
            tc.tile_pool(name="ps", bufs=2, space="PSUM") as psp,
            tc.tile_pool(name="idx", bufs=2) as idxp,
            tc.tile_pool(name="sm", bufs=2) as sm,
            tc.tile_pool(name="out", bufs=2) as outp,
        ):
            iotam_sb = cst.tile([128, 4], FP8)
            nc.gpsimd.dma_start(iotam_sb[:], iotam_d[:])
            iota_sb = cst.tile([128, 256], BF16)
            nc.gpsimd.dma_start(iota_sb[:], iota_d[:])

            for t in [t for _ in range(reps) for t in range(n_iter)]:
                a_t = ld.tile([128, 8, rows_per_iter // 128 * 128], FP8, tag="a")
                nc.sync.dma_start(
                    a_t[:], a_v[:, :, t * rows_per_iter : (t + 1) * rows_per_iter]
                    .rearrange("pl r w -> r pl w")
                )
                b_t = ld.tile([128, 8, rows_per_iter], FP8, tag="b")
                nc.sync.dma_start(
                    b_t[:], b_v[:, :, t * rows_per_iter : (t + 1) * rows_per_iter]
                    .rearrange("pl r w -> r pl w")
                )

                # psum: [word_p, wchunk, tensor, byte, nib]
                ps = psp.tile([128, w, 2, 4, 2], F32, tag="ps")
                for k in range(w):
                    for ti, src in enumerate((a_t, b_t)):
                        for byte in range(4):
                            for h in range(2):
                                nc.tensor.matmul(
                                    ps[:, k, ti, byte, :],
                                    src[:, byte * 2 + h, k * 128 : (k + 1) * 128],
                                    iotam_sb[:, 2 * h : 2 * h + 2],
                                    start=(h == 0),
                                    stop=(h == 1),
                                )

                # evacuate psum once, then byte index = lo + 16*hi per tensor
                nib = idxp.tile([128, w, 2, 4, 2], F32, tag="nib")
                nc.vector.tensor_copy(nib[:], ps[:])
                idxa = idxp.tile([128, nseg], F32, tag="ia")
                nc.vector.scalar_tensor_tensor(
                    idxa[:].rearrange("p (k i) -> p k i", i=4),
                    nib[:, :, 0, :, 1], 16.0, nib[:, :, 0, :, 0],
                    OP.mult, OP.add,
                )
                idxb = idxp.tile([128, nseg], F32, tag="ib")
                nc.vector.scalar_tensor_tensor(
                    idxb[:].rearrange("p (k i) -> p k i", i=4),
                    nib[:, :, 1, :, 1], 16.0, nib[:, :, 1, :, 0],
                    OP.mult, OP.add,
                )

                # ripple-carry add over byte positions i=0..3 (i inner in col)
                def bslice(ap, i):
                    return ap.rearrange("p (s i) -> p i s", i=4)[:, i : i + 1, :]

                csum = idxp.tile([128, nseg], F32, tag="cs")
                carry = None
                for i in range(4):
                    t0 = sm.tile([128, w], F32, tag=f"t0{i}")
                    nc.vector.tensor_tensor(
                        t0[:].rearrange("p (i s) -> p i s", i=1),
                        bslice(idxa[:], i),
                        bslice(idxb[:], i),
                        OP.add,
                    )
                    if carry is not None:
                        nc.vector.tensor_tensor(t0[:], t0[:], carry[:], OP.add)
                    cnew = sm.tile([128, w], F32, tag=f"c{i}")
                    nc.vector.tensor_scalar(cnew[:], t0[:], 256.0, None, OP.is_ge)
                    nc.vector.scalar_tensor_tensor(
                        bslice(csum[:], i),
                        cnew[:].rearrange("p (i s) -> p i s", i=1),
                        -256.0,
                        t0[:].rearrange("p (i s) -> p i s", i=1),
                        OP.mult,
                        OP.add,
                    )
                    carry = cnew

                # xor with operand a (int32), back to f32 for compares
                s_i = sm.tile([128, nseg], I32, tag="si")
                nc.vector.tensor_copy(s_i[:], csum[:])
                a_i = sm.tile([128, nseg], I32, tag="ai")
                nc.vector.tensor_copy(a_i[:], idxa[:])
                x_i = sm.tile([128, nseg], I32, tag="xi")
                nc.vector.tensor_tensor(x_i[:], s_i[:], a_i[:], OP.bitwise_xor)
                x_f = sm.tile([128, nseg], F32, tag="xf")
                nc.vector.tensor_copy(x_f[:], x_i[:])

                # DVE: direct is_equal (bf16) for wchunks [0, dve_w).
                out_t = outp.tile([128, dve_w * 1024], BF16, tag="o")
                for j in range(4 * dve_w):
                    nc.vector.tensor_scalar(
                        out_t[:, j * 256 : (j + 1) * 256],
                        iota_sb[:, 0:256],
                        x_f[:, j : j + 1],
                        None,
                        OP.is_equal,
                    )
                nc.gpsimd.dma_start(
                    y_v[t][:, 0:dve_w],
                    out_t[:].rearrange("p (s c) -> p s c", c=1024),
                )
                # ACT wchunks: DVE prediff d = iota - x per seg, then one
                # whole-region Square and one Relu(1-sq) -> fp8 on ACT.
                if act_chunks:
                    d_t = outp.tile([128, act_chunks * 1024], BF16, tag="d")
                    for j in range(4 * dve_w, nseg):
                        jj = j - 4 * dve_w
                        nc.vector.tensor_scalar(
                            d_t[:, jj * 256 : (jj + 1) * 256],
                            iota_sb[:, 0:256],
                            x_f[:, j : j + 1],
                            None,
                            OP.subtract,
                        )
                    sq_t = outp.tile([128, act_chunks * 1024], BF16, tag="sq")
                    nc.scalar.activation(sq_t[:], d_t[:], ACT.Square)
                    oa_t = outp.tile([128, act_chunks * 1024], FP8, tag="oa")
                    nc.scalar.activation(
                        oa_t[:], sq_t[:], ACT.Relu, bias=1.0, scale=-1.0
                    )
                    nc.sync.dma_start(
                        y_v[t][:, dve_w:w],
                        oa_t[:].rearrange("p (s c) -> p s c", c=1024),
                    )

    nc.compile()
    return nc


_CACHED = {}


def _get_kernel(n_words=BLOC, w=W):
    key = (n_words, w)
    if key not in _CACHED:
        _CACHED[key] = build_kernel(n_words, w)
    return _CACHED[key]


def _iotam_tile():
    # cols [lo(h0), hi(h0), lo(h1), hi(h1)] for c = 128*h + r
    r = np.arange(128, dtype=np.float32)
    c0 = r
    c1 = 128 + r
    m = np.stack(
        [c0 % 16, c0 // 16, c1 % 16, c1 // 16], axis=1
    ).astype(np.float32)
    return m.astype(ml_dtypes.float8_e4m3)


def _iota256_tile():
    row = np.arange(256, dtype=np.float32)
    return np.broadcast_to(row, (128, 256)).astype(ml_dtypes.bfloat16)


def make_in_maps(a, b, w=W):
    """a, b: [B, 1024] float arrays -> per-core input dicts (transposed fp8)."""
    iotam = _iotam_tile()
    iota = _iota256_tile()
    maps = []
    for c in range(NCORES):
        asl = np.asarray(a[c * BLOC : (c + 1) * BLOC]).astype(ml_dtypes.float8_e4m3)
        bsl = np.asarray(b[c * BLOC : (c + 1) * BLOC]).astype(ml_dtypes.float8_e4m3)
        maps.append(
            {
                "a": np.ascontiguousarray(asl.T),
                "b": np.ascontiguousarray(bsl.T),
                "iotam": iotam,
                "iota": iota,
            }
        )
    return maps


def kernel(**inputs):
    a = np.asarray(inputs["a_bytes"], dtype=np.float32).reshape(B, 1024)
    b = np.asarray(inputs["b_bytes"], dtype=np.float32).reshape(B, 1024)
    nc = _get_kernel()
    in_maps = make_in_maps(a, b)
    res = bass_utils.run_bass_kernel_spmd(nc, in_maps, core_ids=list(range(NCORES)))
    out = np.concatenate(
        [res.results[c]["y"].astype(np.float32) for c in range(NCORES)], axis=0
    )
    return out.reshape(B, 4, 256)
